# revision 5
# baseline (speedup 1.0000x reference)
"""Trainium2 Bass kernel for CubePadding (p=1) on x:[6,128,512,512] f32.

Sharding: channel dim C=128 split across 8 NeuronCores (16 channels each).
Each core pads its channel slice independently — zero communication.

Per core the op is pure data movement:
  out[f, :, 1:H+1, 1:W+1] = x[f]               (interior, ~100 MB)
  + 4 halo strips per face gathered from neighbor faces (rows/cols,
    some reversed) + 4 corner pixels per face.
All of it is expressed as DRAM->DRAM DMAs; strips with non-contiguous
last dims fall back to per-element descriptors (allowed explicitly).
"""

import os
import sys

import numpy as np

for _p in (
    "/root/.axon_site",
    "/root/.axon_site/_ro/trn_rl_repo",
    "/root/.axon_site/_ro/pypackages",
    "/opt/trn_rl_repo",
    "/opt/pypackages",
):
    if os.path.isdir(_p) and _p not in sys.path:
        sys.path.append(_p)

N_CORES = 8
FACES, C, H, W = 6, 128, 512, 512
CL = C // N_CORES  # channels per core

# Halo source tables: per output face, a slice of the local input x
# ([6, CL, h, w]) giving the [CL, w] strip. Derived from reference
# _cube_pad with p=1; validated exactly in mapping_check.py. Bass APs
# don't support negative indices, so h/w are threaded explicitly.
T_SRC = [
    lambda x, h, w: x[5][:, 0, ::-1],
    lambda x, h, w: x[2][:, h - 1, :],
    lambda x, h, w: x[5][:, h - 1, :],
    lambda x, h, w: x[5][:, :, 0],
    lambda x, h, w: x[5][:, h - 1 :: -1, w - 1],
    lambda x, h, w: x[0][:, 0, ::-1],
]
D_SRC = [
    lambda x, h, w: x[1][:, h - 1, ::-1],
    lambda x, h, w: x[0][:, h - 1, ::-1],
    lambda x, h, w: x[1][:, 0, :],
    lambda x, h, w: x[1][:, h - 1 :: -1, 0],
    lambda x, h, w: x[1][:, :, w - 1],
    lambda x, h, w: x[2][:, 0, :],
]
L_SRC = [
    lambda x, h, w: x[4][:, :, w - 1],
    lambda x, h, w: x[3][:, h - 1, ::-1],
    lambda x, h, w: x[3][:, :, w - 1],
    lambda x, h, w: x[0][:, :, w - 1],
    lambda x, h, w: x[2][:, :, w - 1],
    lambda x, h, w: x[3][:, 0, :],
]
R_SRC = [
    lambda x, h, w: x[3][:, :, 0],
    lambda x, h, w: x[4][:, h - 1, :],
    lambda x, h, w: x[4][:, :, 0],
    lambda x, h, w: x[2][:, :, 0],
    lambda x, h, w: x[0][:, :, 0],
    lambda x, h, w: x[4][:, 0, ::-1],
]


def build_nc(cl=CL, h=H, w=W):
    """Build the per-core Bass program (same NEFF on every core).

    Raw Block (no Tile): pure-DMA kernel, one semaphore, one final wait.
    Interiors go on the sync (SP) HWDGE ring; halo strips + corners on
    the scalar (ACT) ring so their descriptor processing overlaps the
    bulk interior transfers.
    """
    from concourse import bass, mybir

    nc = bass.Bass()
    x = nc.declare_dram_parameter(
        "x", [FACES, cl, h, w], mybir.dt.float32, isOutput=False
    )
    out = nc.declare_dram_parameter(
        "out", [FACES, cl, h + 2, w + 2], mybir.dt.float32, isOutput=True
    )

    n_total = FACES + FACES * 8  # 6 interiors + 8 halo DMAs per face

    with nc.Block() as block, nc.semaphore("dma_sem") as dma_sem:

        @block.scalar
        def _(scalar: "bass.BassEngine"):
            with nc.allow_non_contiguous_dma(reason="cube-pad halo gathers"):
                for f in range(FACES):
                    t = T_SRC[f](x, h, w)
                    d = D_SRC[f](x, h, w)
                    l = L_SRC[f](x, h, w)
                    r = R_SRC[f](x, h, w)
                    for dst, src in (
                        (out[f, :, 0, 1 : w + 1], t),
                        (out[f, :, h + 1, 1 : w + 1], d),
                        (out[f, :, 1 : h + 1, 0], l),
                        (out[f, :, 1 : h + 1, w + 1], r),
                        # corners: p=1 degenerates to first/last strip elem
                        (out[f, :, 0, 0:1], t[:, 0:1]),
                        (out[f, :, 0, w + 1 : w + 2], t[:, w - 1 : w]),
                        (out[f, :, h + 1, 0:1], d[:, 0:1]),
                        (out[f, :, h + 1, w + 1 : w + 2], d[:, w - 1 : w]),
                    ):
                        scalar.dma_start(out=dst, in_=src).then_inc(dma_sem, 16)

        @block.sync
        def _(sync: "bass.BassEngine"):
            for f in range(FACES):
                sync.dma_start(
                    out=out[f, :, 1 : h + 1, 1 : w + 1], in_=x[f]
                ).then_inc(dma_sem, 16)
            sync.wait_ge(dma_sem, 16 * n_total)

    return nc


_built_nc = None

# Set TRACE=True (e.g. from test.py) to capture an NTFF profile; the
# BassKernelResults of the last run land in LAST_RESULTS.
TRACE = False
LAST_RESULTS = None


def kernel(x, lrtd_pad):
    global _built_nc, LAST_RESULTS
    p = int(lrtd_pad)
    assert p == 1, f"kernel hardcodes p=1, got {p}"
    x = np.asarray(x, dtype=np.float32)
    assert x.shape == (FACES, C, H, W), x.shape

    from concourse.bass_utils import run_bass_kernel_spmd

    if _built_nc is None:
        _built_nc = build_nc()

    in_maps = [
        {"x": np.ascontiguousarray(x[:, i * CL : (i + 1) * CL])}
        for i in range(N_CORES)
    ]
    res = run_bass_kernel_spmd(
        _built_nc, in_maps, list(range(N_CORES)), trace=TRACE
    )
    LAST_RESULTS = res
    return np.concatenate([r["out"] for r in res.results], axis=1)


# revision 10
# speedup vs baseline: 2.3587x; 2.3587x over previous
"""Trainium2 Bass kernel for CubePadding (p=1) on x:[6,128,512,512] f32.

Sharding: channel dim C=128 split across 8 NeuronCores (16 channels each).
Each core pads its channel slice independently — zero communication.

Per core the op is pure data movement:
  out[f, :, 1:H+1, 1:W+1] = x[f]               (interior, ~100 MB)
  + 4 halo strips per face gathered from neighbor faces (rows/cols,
    some reversed) + 4 corner pixels per face.
All of it is expressed as DRAM->DRAM DMAs; strips with non-contiguous
last dims fall back to per-element descriptors (allowed explicitly).
"""

import os
import sys

import numpy as np

for _p in (
    "/root/.axon_site",
    "/root/.axon_site/_ro/trn_rl_repo",
    "/root/.axon_site/_ro/pypackages",
    "/opt/trn_rl_repo",
    "/opt/pypackages",
):
    if os.path.isdir(_p) and _p not in sys.path:
        sys.path.append(_p)

N_CORES = 8
FACES, C, H, W = 6, 128, 512, 512
CL = C // N_CORES  # channels per core

# Halo source tables: per output face, a slice of the local input x
# ([6, CL, h, w]) giving the [CL, w] strip. Derived from reference
# _cube_pad with p=1; validated exactly in mapping_check.py. Bass APs
# don't support negative indices, so h/w are threaded explicitly.
T_SRC = [
    lambda x, h, w: x[5][:, 0, ::-1],
    lambda x, h, w: x[2][:, h - 1, :],
    lambda x, h, w: x[5][:, h - 1, :],
    lambda x, h, w: x[5][:, :, 0],
    lambda x, h, w: x[5][:, h - 1 :: -1, w - 1],
    lambda x, h, w: x[0][:, 0, ::-1],
]
D_SRC = [
    lambda x, h, w: x[1][:, h - 1, ::-1],
    lambda x, h, w: x[0][:, h - 1, ::-1],
    lambda x, h, w: x[1][:, 0, :],
    lambda x, h, w: x[1][:, h - 1 :: -1, 0],
    lambda x, h, w: x[1][:, :, w - 1],
    lambda x, h, w: x[2][:, 0, :],
]
L_SRC = [
    lambda x, h, w: x[4][:, :, w - 1],
    lambda x, h, w: x[3][:, h - 1, ::-1],
    lambda x, h, w: x[3][:, :, w - 1],
    lambda x, h, w: x[0][:, :, w - 1],
    lambda x, h, w: x[2][:, :, w - 1],
    lambda x, h, w: x[3][:, 0, :],
]
R_SRC = [
    lambda x, h, w: x[3][:, :, 0],
    lambda x, h, w: x[4][:, h - 1, :],
    lambda x, h, w: x[4][:, :, 0],
    lambda x, h, w: x[2][:, :, 0],
    lambda x, h, w: x[0][:, :, 0],
    lambda x, h, w: x[4][:, 0, ::-1],
]


def build_nc_v1(cl=CL, h=H, w=W):
    """v1 (checkpoint): pure DRAM->DRAM DMAs; halo strips use per-element
    descriptors which turned out to dominate (1.70 ms measured)."""
    from concourse import bass, mybir

    nc = bass.Bass()
    x = nc.declare_dram_parameter(
        "x", [FACES, cl, h, w], mybir.dt.float32, isOutput=False
    )
    out = nc.declare_dram_parameter(
        "out", [FACES, cl, h + 2, w + 2], mybir.dt.float32, isOutput=True
    )

    n_total = FACES + FACES * 8  # 6 interiors + 8 halo DMAs per face

    with nc.Block() as block, nc.semaphore("dma_sem") as dma_sem:

        @block.scalar
        def _(scalar: "bass.BassEngine"):
            with nc.allow_non_contiguous_dma(reason="cube-pad halo gathers"):
                for f in range(FACES):
                    t = T_SRC[f](x, h, w)
                    d = D_SRC[f](x, h, w)
                    l = L_SRC[f](x, h, w)
                    r = R_SRC[f](x, h, w)
                    for dst, src in (
                        (out[f, :, 0, 1 : w + 1], t),
                        (out[f, :, h + 1, 1 : w + 1], d),
                        (out[f, :, 1 : h + 1, 0], l),
                        (out[f, :, 1 : h + 1, w + 1], r),
                        # corners: p=1 degenerates to first/last strip elem
                        (out[f, :, 0, 0:1], t[:, 0:1]),
                        (out[f, :, 0, w + 1 : w + 2], t[:, w - 1 : w]),
                        (out[f, :, h + 1, 0:1], d[:, 0:1]),
                        (out[f, :, h + 1, w + 1 : w + 2], d[:, w - 1 : w]),
                    ):
                        scalar.dma_start(out=dst, in_=src).then_inc(dma_sem, 16)

        @block.sync
        def _(sync: "bass.BassEngine"):
            for f in range(FACES):
                sync.dma_start(
                    out=out[f, :, 1 : h + 1, 1 : w + 1], in_=x[f]
                ).then_inc(dma_sem, 16)
            sync.wait_ge(dma_sem, 16 * n_total)

    return nc


def build_nc(cl=CL, h=H, w=W):
    """v2: SBUF-staged pipeline with big-descriptor DMAs only.

    Per channel c (16 per core), software-pipelined over channels:
      sync:   load x[f,c] -> dense X tiles [P,4,w]      (8 KB descriptors)
      vector: extract halo columns from X ([P,4] DVE); re-layout X -> O
              [P,4,w+2] (interior at col offset 1); batched reversals
      gpsimd: row extracts as 1-descriptor SBUF->SBUF DMAs; bounce the 8
              geometry-crossing strips via DRAM scratch ([P,4] <->
              contiguous row, 16 B descriptors, tiny)
      scalar: store O -> out[f,c,1:h+1,:] as full (w+2)-wide rows
              (8 KB descriptors), plus [1,w] top/bottom row stores
    X double-buffered; O single-buffered with per-face store gating.
    Corner pixels are 24 tiny DRAM->DRAM DMAs (independent).
    Engine (non-DMA) ops only touch partition-0-based tiles (HW limit:
    start partition must be 0/32/64/96); DMAs have no such limit.
    """
    from concourse import bass, mybir

    assert h % 4 == 0
    P = h // 4  # partitions used by X/O tiles (=128 at full size)
    f32 = mybir.dt.float32

    nc = bass.Bass()
    x = nc.declare_dram_parameter("x", [FACES, cl, h, w], f32, isOutput=False)
    out = nc.declare_dram_parameter(
        "out", [FACES, cl, h + 2, w + 2], f32, isOutput=True
    )
    scratch = nc.dram_tensor("scratch", [2, 6, w], f32)

    X = [
        [nc.alloc_sbuf_tensor(f"xb{b}f{f}", [P, 4, w], f32) for f in range(FACES)]
        for b in range(2)
    ]
    O = [nc.alloc_sbuf_tensor(f"of{f}", [P, 4, w + 2], f32) for f in range(FACES)]
    # rows needing reversal, gathered then batch-reversed RAW -> REV:
    # 0=T0(x5 r0) 1=T5(x0 r0) 2=D0(x1 r511) 3=D1(x0 r511)
    # 4=L1(x3 r511) 5=R5(x4 r0)
    RAW = [nc.alloc_sbuf_tensor(f"raw{b}", [6, w], f32) for b in range(2)]
    REV = [nc.alloc_sbuf_tensor(f"rev{b}", [6, w], f32) for b in range(2)]
    # bounced-back T4/D3 rows (reversal pending): RAW2 -> REV2
    RAW2 = [nc.alloc_sbuf_tensor(f"raw2{b}", [2, w], f32) for b in range(2)]
    REV2 = [nc.alloc_sbuf_tensor(f"rev2{b}", [2, w], f32) for b in range(2)]
    # Fout: 0=T3(direct) 1=T4(bounce) 2=D3(bounce) 3=D4(direct)
    Fout = [
        [nc.alloc_sbuf_tensor(f"fo{b}j{j}", [P, 4], f32) for j in range(4)]
        for b in range(2)
    ]
    # Fin: bounced-back [P,4] for row->col strips: 0=L1 1=L5 2=R1 3=R5
    Fin = [
        [nc.alloc_sbuf_tensor(f"fi{b}j{j}", [P, 4], f32) for j in range(4)]
        for b in range(2)
    ]

    # per-channel semaphore increments
    LOAD_INC = 16 * FACES          # 6 loads
    REXT_INC = 16 * 6              # 6 row-extract DMAs
    SCR_OUT_INC = 16 * 8           # 2 direct row stores + 6 scratch-outs
    SCR_IN_INC = 16 * 6            # 6 scratch-ins
    TD_INC = 16 * 10               # 10 top/bottom row stores
    OF_INC = 16                    # 1 O store per face per channel
    N_CORNER = 16 * 4 * FACES      # 24 corner DMAs

    with (
        nc.Block() as block,
        nc.semaphore("load_sem0") as load_sem0,
        nc.semaphore("load_sem1") as load_sem1,
        nc.semaphore("v1_sem") as v1_sem,      # Fout extracts done
        nc.semaphore("v2_sem") as v2_sem,      # 2/channel: v2a, v2b
        nc.semaphore("rext_sem0") as rext_sem0,
        nc.semaphore("rext_sem1") as rext_sem1,
        nc.semaphore("rev1_sem") as rev1_sem,
        nc.semaphore("scr_out_sem") as scr_out_sem,
        nc.semaphore("scr_in_sem") as scr_in_sem,
        nc.semaphore("td_sem") as td_sem,
        nc.semaphore("corner_sem") as corner_sem,
        nc.semaphore("of0") as of0,
        nc.semaphore("of1") as of1,
        nc.semaphore("of2") as of2,
        nc.semaphore("of3") as of3,
        nc.semaphore("of4") as of4,
        nc.semaphore("of5") as of5,
    ):
        ofs = [of0, of1, of2, of3, of4, of5]
        load_sems = [load_sem0, load_sem1]
        rext_sems = [rext_sem0, rext_sem1]

        @block.sync
        def _(sync: "bass.BassEngine"):
            for i in range(cl):
                b = i % 2
                if i >= 2:
                    # X[b] readers of channel i-2: vector O-copies (v2a),
                    # scalar direct row stores (td), gpsimd extracts
                    # (rext) and direct scratch-outs (scr_out)
                    sync.wait_ge(v2_sem, 2 * (i - 2) + 1)
                    sync.wait_ge(td_sem, TD_INC * (i - 1))
                    sync.wait_ge(rext_sems[b], REXT_INC * (i // 2))
                    sync.wait_ge(scr_out_sem, SCR_OUT_INC * (i - 1))
                for f in range(FACES):
                    sync.dma_start(out=X[b][f][:, :, :], in_=x[f, i]).then_inc(
                        load_sems[b], 16
                    )

        @block.vector
        def _(vector: "bass.BassEngine"):
            for i in range(cl):
                b = i % 2
                vector.wait_ge(load_sems[b], LOAD_INC * (i // 2 + 1))
                if i >= 2:
                    # Fout/RAW/REV/REV2 parity reuse
                    vector.wait_ge(td_sem, TD_INC * (i - 1))
                    vector.wait_ge(scr_out_sem, SCR_OUT_INC * (i - 1))
                # col extracts: T3(x5 c0) T4(x5 c511) D3(x1 c0) D4(x1 c511)
                vector.tensor_copy(Fout[b][0][:, :], X[b][5][:, :, 0])
                vector.tensor_copy(Fout[b][1][:, :], X[b][5][:, :, w - 1])
                vector.tensor_copy(Fout[b][2][:, :], X[b][1][:, :, 0])
                vector.tensor_copy(Fout[b][3][:, :], X[b][1][:, :, w - 1]).then_inc(
                    v1_sem, 1
                )
                # O re-layout + halo column injection (per-face gated)
                colcol = {  # dst face -> (dst col, src face, src col)
                    0: [(0, 4, w - 1), (w + 1, 3, 0)],
                    2: [(0, 3, w - 1), (w + 1, 4, 0)],
                    3: [(0, 0, w - 1), (w + 1, 2, 0)],
                    4: [(0, 2, w - 1), (w + 1, 0, 0)],
                }
                last = None
                for f in (1, 5, 0, 2, 3, 4):
                    if i >= 1:
                        vector.wait_ge(ofs[f], OF_INC * i)
                    last = vector.tensor_copy(O[f][:, :, 1 : w + 1], X[b][f][:, :, :])
                    for dcol, sf, scol in colcol.get(f, []):
                        last = vector.tensor_copy(
                            O[f][:, :, dcol], X[b][sf][:, :, scol]
                        )
                last.then_inc(v2_sem, 1)  # v2a: X free for vector
                # batched reversal of gathered rows
                vector.wait_ge(rext_sems[b], REXT_INC * (i // 2 + 1))
                vector.tensor_copy(REV[b][:, :], RAW[b][:, w - 1 :: -1]).then_inc(
                    rev1_sem, 1
                )
                # bounced pieces
                vector.wait_ge(scr_in_sem, SCR_IN_INC * (i + 1))
                vector.tensor_copy(REV2[b][:, :], RAW2[b][:, w - 1 :: -1])
                vector.tensor_copy(O[1][:, :, 0], Fin[b][0][:, :])
                vector.tensor_copy(O[5][:, :, 0], Fin[b][1][:, :])
                vector.tensor_copy(O[1][:, :, w + 1], Fin[b][2][:, :])
                vector.tensor_copy(O[5][:, :, w + 1], Fin[b][3][:, :]).then_inc(
                    v2_sem, 1
                )  # v2b

        @block.gpsimd
        def _(gpsimd: "bass.BassEngine"):
            for i in range(cl):
                b = i % 2
                gpsimd.wait_ge(load_sems[b], LOAD_INC * (i // 2 + 1))
                if i >= 2:
                    gpsimd.wait_ge(rev1_sem, i - 1)       # RAW reuse
                    gpsimd.wait_ge(scr_in_sem, SCR_IN_INC * (i - 1))
                    gpsimd.wait_ge(v2_sem, 2 * (i - 1))   # Fin/RAW2 reuse
                # row extracts into RAW (SBUF->SBUF, 1 desc each)
                for j, (sf, r, rv) in enumerate((
                    (5, 0, 0), (0, 0, 0), (1, h - 1, 0),
                    (0, h - 1, 0), (3, h - 1, 0), (4, 0, 0),
                )):
                    gpsimd.dma_start(
                        out=RAW[b][j : j + 1, :],
                        in_=X[b][sf][r // 4 : r // 4 + 1, r % 4, :],
                    ).then_inc(rext_sems[b], 16)
                gpsimd.wait_ge(v1_sem, i + 1)  # Fout ready
                # direct stores: T3 -> out[3] top row, D4 -> out[4] bottom
                gpsimd.dma_start(
                    out=out[3, i, 0, 1 : w + 1], in_=Fout[b][0][:, :]
                ).then_inc(scr_out_sem, 16)
                gpsimd.dma_start(
                    out=out[4, i, h + 1, 1 : w + 1], in_=Fout[b][3][:, :]
                ).then_inc(scr_out_sem, 16)
                # scratch-outs: T4, D3 cols; L5, R1 rows straight from X
                gpsimd.dma_start(out=scratch[b, 0], in_=Fout[b][1][:, :]).then_inc(
                    scr_out_sem, 16
                )
                gpsimd.dma_start(out=scratch[b, 1], in_=Fout[b][2][:, :]).then_inc(
                    scr_out_sem, 16
                )
                gpsimd.dma_start(
                    out=scratch[b, 3], in_=X[b][3][0:1, 0, :]  # L5 = x3 r0
                ).then_inc(scr_out_sem, 16)
                gpsimd.dma_start(
                    out=scratch[b, 4],
                    in_=X[b][4][P - 1 : P, 3, :],  # R1 = x4 r511
                ).then_inc(scr_out_sem, 16)
                # reversed rows L1, R5 from REV
                gpsimd.wait_ge(rev1_sem, i + 1)
                gpsimd.dma_start(out=scratch[b, 2], in_=REV[b][4:5, :]).then_inc(
                    scr_out_sem, 16
                )
                gpsimd.dma_start(out=scratch[b, 5], in_=REV[b][5:6, :]).then_inc(
                    scr_out_sem, 16
                )
                gpsimd.wait_ge(scr_out_sem, SCR_OUT_INC * (i + 1))
                # previous channel's scr_in batch fully landed (already
                # implied by the v2 wait; explicit for inc-order safety)
                gpsimd.wait_ge(scr_in_sem, SCR_IN_INC * i)
                # scratch-ins: T4/D3 rows to RAW2; L1 L5 R1 R5 cols to Fin
                gpsimd.dma_start(out=RAW2[b][0:1, :], in_=scratch[b, 0]).then_inc(
                    scr_in_sem, 16
                )
                gpsimd.dma_start(out=RAW2[b][1:2, :], in_=scratch[b, 1]).then_inc(
                    scr_in_sem, 16
                )
                for j in range(4):
                    gpsimd.dma_start(
                        out=Fin[b][j][:, :], in_=scratch[b, 2 + j]
                    ).then_inc(scr_in_sem, 16)

        @block.scalar
        def _(scalar: "bass.BassEngine"):
            # corners: p=1 degenerates to single strip elements (DRAM->DRAM)
            with nc.allow_non_contiguous_dma(reason="corner pixels"):
                for f in range(FACES):
                    t = T_SRC[f](x, h, w)
                    d = D_SRC[f](x, h, w)
                    for dst, src in (
                        (out[f, :, 0, 0:1], t[:, 0:1]),
                        (out[f, :, 0, w + 1 : w + 2], t[:, w - 1 : w]),
                        (out[f, :, h + 1, 0:1], d[:, 0:1]),
                        (out[f, :, h + 1, w + 1 : w + 2], d[:, w - 1 : w]),
                    ):
                        scalar.dma_start(out=dst, in_=src).then_inc(corner_sem, 16)
            for i in range(cl):
                b = i % 2
                scalar.wait_ge(v2_sem, 2 * i + 1)  # v2a of channel i
                # previous batches fully landed (implied; inc-order safety)
                scalar.wait_ge(td_sem, TD_INC * i)
                for f in range(FACES):
                    scalar.wait_ge(ofs[f], OF_INC * i)
                for f in (0, 2, 3, 4):
                    scalar.dma_start(
                        out=out[f, i, 1 : h + 1, :], in_=O[f][:, :, :]
                    ).then_inc(ofs[f], 16)
                # non-reversed t/d rows straight from X:
                # T1=x2 r511, T2=x5 r511, D2=x1 r0, D5=x2 r0
                scalar.dma_start(
                    out=out[1, i, 0, 1 : w + 1], in_=X[b][2][P - 1 : P, 3, :]
                ).then_inc(td_sem, 16)
                scalar.dma_start(
                    out=out[2, i, 0, 1 : w + 1], in_=X[b][5][P - 1 : P, 3, :]
                ).then_inc(td_sem, 16)
                scalar.dma_start(
                    out=out[2, i, h + 1, 1 : w + 1], in_=X[b][1][0:1, 0, :]
                ).then_inc(td_sem, 16)
                scalar.dma_start(
                    out=out[5, i, h + 1, 1 : w + 1], in_=X[b][2][0:1, 0, :]
                ).then_inc(td_sem, 16)
                # reversed rows from REV: T0, T5, D0, D1
                scalar.wait_ge(rev1_sem, i + 1)
                scalar.dma_start(
                    out=out[0, i, 0, 1 : w + 1], in_=REV[b][0:1, :]
                ).then_inc(td_sem, 16)
                scalar.dma_start(
                    out=out[5, i, 0, 1 : w + 1], in_=REV[b][1:2, :]
                ).then_inc(td_sem, 16)
                scalar.dma_start(
                    out=out[0, i, h + 1, 1 : w + 1], in_=REV[b][2:3, :]
                ).then_inc(td_sem, 16)
                scalar.dma_start(
                    out=out[1, i, h + 1, 1 : w + 1], in_=REV[b][3:4, :]
                ).then_inc(td_sem, 16)
                scalar.wait_ge(v2_sem, 2 * (i + 1))  # v2b of channel i
                for f in (1, 5):
                    scalar.dma_start(
                        out=out[f, i, 1 : h + 1, :], in_=O[f][:, :, :]
                    ).then_inc(ofs[f], 16)
                # bounced+reversed rows: T4 = REV2[0], D3 = REV2[1]
                scalar.dma_start(
                    out=out[4, i, 0, 1 : w + 1], in_=REV2[b][0:1, :]
                ).then_inc(td_sem, 16)
                scalar.dma_start(
                    out=out[3, i, h + 1, 1 : w + 1], in_=REV2[b][1:2, :]
                ).then_inc(td_sem, 16)
            # final barrier: all output writes complete
            for f in range(FACES):
                scalar.wait_ge(ofs[f], OF_INC * cl)
            scalar.wait_ge(td_sem, TD_INC * cl)
            scalar.wait_ge(scr_out_sem, SCR_OUT_INC * cl)
            scalar.wait_ge(corner_sem, N_CORNER)

    return nc


_built_nc = None

# Set TRACE=True (e.g. from test.py) to capture an NTFF profile; the
# BassKernelResults of the last run land in LAST_RESULTS.
TRACE = False
LAST_RESULTS = None


def kernel(x, lrtd_pad):
    global _built_nc, LAST_RESULTS
    p = int(lrtd_pad)
    assert p == 1, f"kernel hardcodes p=1, got {p}"
    x = np.asarray(x, dtype=np.float32)
    assert x.shape == (FACES, C, H, W), x.shape

    from concourse.bass_utils import run_bass_kernel_spmd

    if _built_nc is None:
        _built_nc = build_nc()

    in_maps = [
        {"x": np.ascontiguousarray(x[:, i * CL : (i + 1) * CL])}
        for i in range(N_CORES)
    ]
    res = run_bass_kernel_spmd(
        _built_nc, in_maps, list(range(N_CORES)), trace=TRACE
    )
    LAST_RESULTS = res
    return np.concatenate([r["out"] for r in res.results], axis=1)
